# revision 1
# baseline (speedup 1.0000x reference)
"""Trainium2 Bass kernel for COTREC-style GNN message passing.

Math (reference):
    cur1 = S @ emb                      (S sparse [N,N], 1M nnz)
    cur2 = S @ cur1
    item = (emb + cur1 + cur2) / 3
    sess = meanpool_sessions(item)      ([B, E])
    ... small dense tail (DA @ ..., w_sess, l2norm) ...

Device decomposition (8 cores, SPMD single program, per-core data via inputs):
  * Row-shard nodes: core m owns rows [m*NLOC, (m+1)*NLOC).
  * cur1 shard computed locally: gather emb[src] per edge (indirect DMA,
    int32 indices, OOB pad slots skipped via bounds_check), segment-sum by
    destination row via mask-matmuls: fixed windows of W rows <-> one
    128-slot tile; mask [128, W] carries the edge values (host-built).
  * cur2 never materialized. The session pooling is pushed through the
    graph: sess*3*len = P01@emb + P01@cur1 + (P01@S)@cur1, where P01/Q01
    structures are host-computed integer index work. Per-core partial
    pooled sums [E, B] -> one small AllReduce.
  * Dense tail replicated on all cores; core 0's output returned.
"""

import os
import sys
from contextlib import ExitStack

import numpy as np

for _p in ("/opt/trn_rl_repo", os.path.expanduser("~/.axon_site/_ro/trn_rl_repo")):
    if os.path.isdir(_p) and _p not in sys.path:
        sys.path.append(_p)

import concourse.bacc as bacc
import concourse.bass as bass
import concourse.tile as tile
from concourse import mybir
from concourse.masks import make_identity

F32 = mybir.dt.float32
I32 = mybir.dt.int32


class Cfg:
    def __init__(self, N=100000, NNZ=1000000, B=512, L=50, M=8, EMB=112, EP=128,
                 W=8, WS=8):
        self.N, self.NNZ, self.B, self.L, self.M = N, NNZ, B, L, M
        self.EMB, self.EP = EMB, EP
        self.W = W            # output rows per L1 window (one 128-slot tile)
        self.WS = WS          # sessions per sess window
        # shard rows per core: multiple of 128, cover N
        self.NLOC = ((N + M - 1) // M + 127) // 128 * 128
        self.G = self.NLOC // 128          # 128-row groups per core
        self.NWIN = self.NLOC // W         # L1 windows per core
        self.TPG = 128 // W                # L1 tiles (windows) per group
        self.NWS = B // WS                 # sess windows
        self.BT = B // 128                 # tail b-tiles


# ---------------------------------------------------------------------------
# Host preprocessing: pure integer/layout work + permutation of input floats.
# ---------------------------------------------------------------------------

def _csr_expand(rowptr, rows):
    """For each r in rows return concatenated [rowptr[r], rowptr[r+1]) ranges."""
    deg = rowptr[rows + 1] - rowptr[rows]
    total = int(deg.sum())
    if total == 0:
        return np.zeros(0, np.int64), deg
    cum = np.cumsum(deg)
    out = np.arange(total, dtype=np.int64) - np.repeat(cum - deg, deg) \
        + np.repeat(rowptr[rows], deg)
    return out, deg


def prep(cfg, inputs):
    """Build per-core input arrays + the (core-independent) program plan."""
    c = cfg
    emb = np.asarray(inputs["embedding"], np.float32)
    av = np.asarray(inputs["adj_vals"], np.float32)
    ar = np.asarray(inputs["adj_rows"], np.int64)
    ac = np.asarray(inputs["adj_cols"], np.int64)
    D = np.asarray(inputs["D"], np.float32)
    A = np.asarray(inputs["A"], np.float32)
    si = np.asarray(inputs["session_item"], np.int64)
    sl = np.asarray(inputs["session_len"], np.float32)
    w_sess = np.asarray(inputs["w_sess"], np.float32)

    # padded embedding table [N, EP]
    emb_pad = np.zeros((c.N, c.EP), np.float32)
    emb_pad[:, :c.EMB] = emb

    # session refs: (b, col) for non-pad items
    b_ref = np.repeat(np.arange(c.B, dtype=np.int64), c.L)
    it_ref = si.ravel()
    keep = it_ref > 0
    b_ref, col_ref = b_ref[keep], it_ref[keep] - 1       # cols in [0, N)

    # CSR of S by row (for Q01 = P01 @ S)
    order = np.argsort(ar, kind="stable")
    ar_s, ac_s, av_s = ar[order], ac[order], av[order]
    rowptr = np.searchsorted(ar_s, np.arange(c.N + 1)).astype(np.int64)
    epos, deg = _csr_expand(rowptr, col_ref)
    q_b = np.repeat(b_ref, deg)
    q_c = ac_s[epos]
    q_v = av_s[epos]

    # prune L1 edges to rows actually referenced by T1/T2
    ref_mask = np.zeros(c.N, bool)
    ref_mask[col_ref] = True
    ref_mask[q_c] = True
    ekeep = ref_mask[ar]
    er, ec, ev = ar[ekeep], ac[ekeep], av[ekeep]

    own_e = er // c.NLOC
    own_ref = col_ref // c.NLOC
    own_q = q_c // c.NLOC

    # ---- L1 per-core window fill ------------------------------------------
    def l1_core(m):
        sel = own_e == m
        r = er[sel] - m * c.NLOC
        cc, vv = ec[sel], ev[sel]
        w = r // c.W
        o = r - w * c.W
        so = np.argsort(w, kind="stable")
        w, o, cc, vv = w[so], o[so], cc[so], vv[so]
        cnt = np.bincount(w, minlength=c.NWIN)
        starts = np.zeros(c.NWIN, np.int64)
        starts[1:] = np.cumsum(cnt)[:-1]
        slot = np.arange(len(w)) - starts[w]
        return w, o, cc, vv, slot, cnt

    l1 = [l1_core(m) for m in range(c.M)]
    l1_max = max(int(x[5].max()) if x[5].size else 0 for x in l1)
    if l1_max > 128:
        raise RuntimeError(f"L1 window overflow: {l1_max} > 128; reduce W")

    l1_idx = np.full((c.M, 128, c.NWIN), c.N, np.int32)      # OOB pad = N
    l1_msk = np.zeros((c.M, 128, c.NWIN, c.W), np.float32)
    for m, (w, o, cc, vv, slot, _) in enumerate(l1):
        l1_idx[m, slot, w] = cc
        l1_msk[m, slot, w, o] = vv

    # ---- sess stream A: P01 @ emb (gathers emb at col_ref) ----------------
    # ---- sess stream B: (P01 + Q01) @ cur1 (gathers local cur1 rows) ------
    def sess_core_stream(m, bb, cc_local, vv):
        w = bb // c.WS
        o = bb - w * c.WS
        so = np.argsort(w, kind="stable")
        w, o, cc_local, vv = w[so], o[so], cc_local[so], vv[so]
        cnt = np.bincount(w, minlength=c.NWS)
        return w, o, cc_local, vv, cnt

    sa = []
    sb = []
    for m in range(c.M):
        selr = own_ref == m
        sa.append(sess_core_stream(m, b_ref[selr], col_ref[selr],
                                   np.ones(int(selr.sum()), np.float32)))
        selq = own_q == m
        bb = np.concatenate([b_ref[selr], q_b[selq]])
        cl = np.concatenate([col_ref[selr] - m * c.NLOC, q_c[selq] - m * c.NLOC])
        vv = np.concatenate([np.ones(int(selr.sum()), np.float32), q_v[selq]])
        sb.append(sess_core_stream(m, bb, cl, vv))

    def stream_caps(streams):
        cnts = np.stack([s[4] for s in streams])       # [M, NWS]
        mx = cnts.max(axis=0)
        caps = np.maximum(128, ((mx + 127) // 128) * 128).astype(np.int64)
        return caps

    sa_caps = stream_caps(sa)
    sb_caps = stream_caps(sb)

    def stream_fill(streams, caps, oob):
        tiles_w = caps // 128                           # tiles per window
        tbase = np.zeros(c.NWS, np.int64)
        tbase[1:] = np.cumsum(tiles_w)[:-1]
        T = int(tiles_w.sum())
        wmap = np.zeros(T, np.int64)
        for wi in range(c.NWS):
            wmap[tbase[wi]:tbase[wi] + tiles_w[wi]] = wi
        idx = np.full((c.M, 128, T), oob, np.int32)
        msk = np.zeros((c.M, 128, T, c.WS), np.float32)
        for m, (w, o, cl, vv, cnt) in enumerate(streams):
            starts = np.zeros(c.NWS, np.int64)
            starts[1:] = np.cumsum(cnt)[:-1]
            slot = np.arange(len(w)) - starts[w]        # slot within window
            t = tbase[w] + slot // 128
            p = slot % 128
            idx[m, p, t] = cl
            msk[m, p, t, o] = vv
        return idx, msk, wmap, T

    sa_idx, sa_msk, sa_wmap, saT = stream_fill(sa, sa_caps, c.N)
    sb_idx, sb_msk, sb_wmap, sbT = stream_fill(sb, sb_caps, c.NLOC)

    # session_len layout for per-partition scale: lenr[p, i] = len[128*i + p]
    lenr = sl.reshape(c.BT, 128).T.astype(np.float32).copy()

    wt = np.stack([w_sess[i].T for i in range(w_sess.shape[0])]).copy()

    in_maps = []
    for m in range(c.M):
        in_maps.append({
            "emb": emb_pad,
            "l1_idx": np.ascontiguousarray(l1_idx[m]),
            "l1_msk": np.ascontiguousarray(l1_msk[m]),
            "sa_idx": np.ascontiguousarray(sa_idx[m]),
            "sa_msk": np.ascontiguousarray(sa_msk[m]),
            "sb_idx": np.ascontiguousarray(sb_idx[m]),
            "sb_msk": np.ascontiguousarray(sb_msk[m]),
            "dt": np.ascontiguousarray(D.T),
            "a": np.ascontiguousarray(A),
            "wt": wt,
            "lenr": lenr,
        })

    plan = {"saT": saT, "sbT": sbT,
            "sa_wmap": sa_wmap.tolist(), "sb_wmap": sb_wmap.tolist()}
    return plan, in_maps


# ---------------------------------------------------------------------------
# Bass program (identical on all cores; per-core behavior comes from inputs)
# ---------------------------------------------------------------------------

CHUNK_TILES = 32


def _chunks(total, size):
    out = []
    s = 0
    while s < total:
        out.append((s, min(size, total - s)))
        s += size
    return out


def build_program(cfg, plan):
    c = cfg
    nc = bacc.Bacc("TRN2", target_bir_lowering=False, debug=False,
                   num_devices=c.M)

    emb_t = nc.dram_tensor("emb", [c.N, c.EP], F32, kind="ExternalInput")
    l1_idx_t = nc.dram_tensor("l1_idx", [128, c.NWIN], I32, kind="ExternalInput")
    l1_msk_t = nc.dram_tensor("l1_msk", [128, c.NWIN, c.W], F32, kind="ExternalInput")
    sa_idx_t = nc.dram_tensor("sa_idx", [128, plan["saT"]], I32, kind="ExternalInput")
    sa_msk_t = nc.dram_tensor("sa_msk", [128, plan["saT"], c.WS], F32, kind="ExternalInput")
    sb_idx_t = nc.dram_tensor("sb_idx", [128, plan["sbT"]], I32, kind="ExternalInput")
    sb_msk_t = nc.dram_tensor("sb_msk", [128, plan["sbT"], c.WS], F32, kind="ExternalInput")
    dt_t = nc.dram_tensor("dt", [c.B, c.B], F32, kind="ExternalInput")
    a_t = nc.dram_tensor("a", [c.B, c.B], F32, kind="ExternalInput")
    wt_t = nc.dram_tensor("wt", [2, c.EMB, c.EMB], F32, kind="ExternalInput")
    lenr_t = nc.dram_tensor("lenr", [128, c.BT], F32, kind="ExternalInput")
    out_t = nc.dram_tensor("out", [c.B, c.EMB], F32, kind="ExternalOutput")

    cur1_t = nc.dram_tensor("cur1", [c.NLOC, c.EP], F32, kind="Internal")
    ar_in_t = nc.dram_tensor("ar_in", [c.EMB, c.B], F32, kind="Internal")
    ar_out_t = nc.dram_tensor("ar_out", [c.EMB, c.B], F32, kind="Internal",
                              addr_space="Shared")

    with tile.TileContext(nc) as tc, ExitStack() as ctx:
        _body(ctx, tc, c, plan, emb_t, l1_idx_t, l1_msk_t, sa_idx_t, sa_msk_t,
              sb_idx_t, sb_msk_t, dt_t, a_t, wt_t, lenr_t, out_t, cur1_t,
              ar_in_t, ar_out_t)

    nc.compile()
    return nc


def _body(ctx, tc, c, plan, emb_t, l1_idx_t, l1_msk_t, sa_idx_t, sa_msk_t,
          sb_idx_t, sb_msk_t, dt_t, a_t, wt_t, lenr_t, out_t, cur1_t,
          ar_in_t, ar_out_t):
    nc = tc.nc
    CT = CHUNK_TILES

    const_p = ctx.enter_context(tc.tile_pool(name="const", bufs=1))
    ident = const_p.tile([128, 128], F32)
    make_identity(nc, ident[:])

    # persistent gather chunk buffers (manual double buffer, memset once so
    # OOB-skipped slots never expose NaN garbage to the matmul)
    gb_p = ctx.enter_context(tc.tile_pool(name="gbuf", bufs=1))
    gbufs = [gb_p.tile([128, CT, c.EP], F32, tag=f"gb{i}", name=f"gb{i}")
             for i in range(2)]
    for t in gbufs:
        nc.vector.memset(t[:], 0.0)

    sess_ps_p = ctx.enter_context(tc.tile_pool(name="sessps", bufs=1, space="PSUM"))
    sess_ps = sess_ps_p.tile([c.EMB, c.B], F32)

    gb_i = 0

    # ---------------- phase 1: cur1 = S @ emb (local row shard) ------------
    with tc.tile_pool(name="l1mi", bufs=3) as mi_p, \
         tc.tile_pool(name="l1ps", bufs=3, space="PSUM") as ps_p, \
         tc.tile_pool(name="l1tp", bufs=2, space="PSUM") as tp_p, \
         tc.tile_pool(name="l1st", bufs=3) as st_p:
        for ch_start, ch_n in _chunks(c.NWIN, CT):
            idx_sb = mi_p.tile([128, CT], I32, tag="idx")
            nc.sync.dma_start(idx_sb[:, :ch_n],
                              l1_idx_t[:, ch_start:ch_start + ch_n])
            msk_sb = mi_p.tile([128, CT, c.W], F32, tag="msk")
            nc.sync.dma_start(msk_sb[:, :ch_n, :],
                              l1_msk_t[:, ch_start:ch_start + ch_n, :])
            gb = gbufs[gb_i % 2]
            gb_i += 1
            for tt in range(ch_n):
                nc.gpsimd.indirect_dma_start(
                    out=gb[:, tt, :], out_offset=None,
                    in_=emb_t[:],
                    in_offset=bass.IndirectOffsetOnAxis(
                        ap=idx_sb[:, tt:tt + 1], axis=0),
                    bounds_check=c.N - 1, oob_is_err=False)

            assert ch_start % c.TPG == 0 and ch_n % c.TPG == 0
            for gl in range(ch_n // c.TPG):
                g = (ch_start // c.TPG) + gl
                ps = ps_p.tile([c.EMB, 128], F32, tag="ps")
                for j in range(c.TPG):
                    t = gl * c.TPG + j
                    nc.tensor.matmul(
                        out=ps[:, j * c.W:(j + 1) * c.W],
                        lhsT=gb[:, t, :c.EMB],
                        rhs=msk_sb[:, t, :],
                        start=(j == 0), stop=(j == c.TPG - 1))
                s1 = st_p.tile([c.EMB, 128], F32, tag="s1")
                nc.vector.tensor_copy(s1[:], ps[:])
                ps2 = tp_p.tile([128, c.EMB], F32, tag="tp")
                nc.tensor.transpose(out=ps2[:], in_=s1[:],
                                    identity=ident[:c.EMB, :c.EMB])
                s2 = st_p.tile([128, c.EP], F32, tag="s2")
                nc.vector.memset(s2[:, c.EMB:], 0.0)
                nc.vector.tensor_copy(s2[:, :c.EMB], ps2[:])
                nc.sync.dma_start(cur1_t[g * 128:(g + 1) * 128, :], s2[:])

    # ---------------- phase 2: pooled partial sums into sess_ps ------------
    streams = [
        (sa_idx_t, sa_msk_t, plan["saT"], plan["sa_wmap"], emb_t, c.N),
        (sb_idx_t, sb_msk_t, plan["sbT"], plan["sb_wmap"], cur1_t, c.NLOC),
    ]
    n_sess_mm = plan["saT"] + plan["sbT"]
    mm_i = 0
    with tc.tile_pool(name="smi", bufs=3) as mi_p:
        for idx_t_, msk_t_, T, wmap, src_t, nrows in streams:
            for ch_start, ch_n in _chunks(T, CT):
                idx_sb = mi_p.tile([128, CT], I32, tag="idx")
                nc.sync.dma_start(idx_sb[:, :ch_n],
                                  idx_t_[:, ch_start:ch_start + ch_n])
                msk_sb = mi_p.tile([128, CT, c.WS], F32, tag="msk")
                nc.sync.dma_start(msk_sb[:, :ch_n, :],
                                  msk_t_[:, ch_start:ch_start + ch_n, :])
                gb = gbufs[gb_i % 2]
                gb_i += 1
                for tt in range(ch_n):
                    nc.gpsimd.indirect_dma_start(
                        out=gb[:, tt, :], out_offset=None,
                        in_=src_t[:],
                        in_offset=bass.IndirectOffsetOnAxis(
                            ap=idx_sb[:, tt:tt + 1], axis=0),
                        bounds_check=nrows - 1, oob_is_err=False)
                for t in range(ch_n):
                    w = wmap[ch_start + t]
                    nc.tensor.matmul(
                        out=sess_ps[:, w * c.WS:(w + 1) * c.WS],
                        lhsT=gb[:, t, :c.EMB],
                        rhs=msk_sb[:, t, :],
                        start=(mm_i == 0), stop=(mm_i == n_sess_mm - 1))
                    mm_i += 1

    # ---------------- phase 3: AllReduce + dense tail ----------------------
    with tc.tile_pool(name="tail", bufs=1) as tp, \
         tc.tile_pool(name="tailps", bufs=1, space="PSUM") as tps, \
         tc.tile_pool(name="tailps2", bufs=1, space="PSUM") as tps2, \
         tc.tile_pool(name="tmp", bufs=2) as tmp_p:
        sess_sb = tp.tile([c.EMB, c.B], F32, tag="sess_sb")
        nc.vector.tensor_copy(sess_sb[:], sess_ps[:])
        nc.sync.dma_start(ar_in_t[:], sess_sb[:])
        nc.gpsimd.collective_compute(
            "AllReduce", mybir.AluOpType.add,
            replica_groups=[list(range(c.M))],
            ins=[ar_in_t.ap().opt()], outs=[ar_out_t.ap().opt()])
        sess_all = tp.tile([c.EMB, c.B], F32, tag="sess_all")
        nc.sync.dma_start(sess_all[:], ar_out_t[:])

        lr = tp.tile([128, c.BT], F32, tag="lr")
        nc.sync.dma_start(lr[:], lenr_t[:])
        rc3 = tp.tile([128, c.BT], F32, tag="rc3")
        nc.vector.reciprocal(rc3[:], lr[:])
        nc.vector.tensor_scalar_mul(rc3[:], rc3[:], 1.0 / 3.0)

        # sess0 b-tiles (scaled) + acc + back-transpose to e-layout
        accs = [tp.tile([128, c.EMB], F32, tag=f"acc{i}", name=f"acc{i}")
                for i in range(c.BT)]
        cur_e = tp.tile([c.EMB, c.B], F32, tag="cur_e0")
        for i in range(c.BT):
            pst = tps.tile([128, c.EMB], F32, tag="tp")
            nc.tensor.transpose(out=pst[:],
                                in_=sess_all[:, i * 128:(i + 1) * 128],
                                identity=ident[:c.EMB, :c.EMB])
            s0 = tmp_p.tile([128, c.EMB], F32, tag="s0")
            nc.scalar.mul(s0[:], pst[:], rc3[:, i:i + 1])
            nc.vector.tensor_copy(accs[i][:], s0[:])
            pse = tps2.tile([c.EMB, 128], F32, tag="tpe")
            nc.tensor.transpose(out=pse[:], in_=s0[:], identity=ident[:, :])
            nc.vector.tensor_copy(cur_e[:, i * 128:(i + 1) * 128], pse[:])

        # DA^T b'-tiles: DA_T[b',b] = sum_k A[k,b'] * D^T[k,b]
        dt_sb = [tp.tile([128, c.B], F32, tag=f"dt{k}", name=f"dtsb{k}")
                 for k in range(c.BT)]
        a_sb = [tp.tile([128, c.B], F32, tag=f"a{k}", name=f"asb{k}")
                for k in range(c.BT)]
        for k in range(c.BT):
            nc.sync.dma_start(dt_sb[k][:], dt_t[k * 128:(k + 1) * 128, :])
            nc.sync.dma_start(a_sb[k][:], a_t[k * 128:(k + 1) * 128, :])
        da_sb = [tp.tile([128, c.B], F32, tag=f"da{i}", name=f"dasb{i}")
                 for i in range(c.BT)]
        for i in range(c.BT):
            ps = tps.tile([128, c.B], F32, tag="daps")
            for k in range(c.BT):
                nc.tensor.matmul(out=ps[:],
                                 lhsT=a_sb[k][:, i * 128:(i + 1) * 128],
                                 rhs=dt_sb[k][:],
                                 start=(k == 0), stop=(k == c.BT - 1))
            nc.vector.tensor_copy(da_sb[i][:], ps[:])

        wt_sb = tp.tile([c.EMB, 2, c.EMB], F32, tag="wt")
        for i in range(2):
            nc.sync.dma_start(wt_sb[:, i, :], wt_t[i])

        for layer in range(2):
            psy = tps.tile([c.EMB, c.B], F32, tag="ypsum")
            nc.tensor.matmul(out=psy[:], lhsT=wt_sb[:, layer, :], rhs=cur_e[:],
                             start=True, stop=True)
            y_e = tmp_p.tile([c.EMB, c.B], F32, tag="y_e")
            nc.vector.tensor_copy(y_e[:], psy[:])
            y_b = []
            for bt in range(c.BT):
                pst = tps.tile([128, c.EMB], F32, tag="tp")
                nc.tensor.transpose(out=pst[:],
                                    in_=y_e[:, bt * 128:(bt + 1) * 128],
                                    identity=ident[:c.EMB, :c.EMB])
                yb = tmp_p.tile([128, c.EMB], F32, tag=f"yb{bt}")
                nc.vector.tensor_copy(yb[:], pst[:])
                y_b.append(yb)
            if layer == 0:
                cur_e = tp.tile([c.EMB, c.B], F32, tag="cur_e1")
            for bt in range(c.BT):
                psz = tps.tile([128, c.EMB], F32, tag="zps")
                for k in range(c.BT):
                    nc.tensor.matmul(out=psz[:],
                                     lhsT=da_sb[k][:, bt * 128:(bt + 1) * 128],
                                     rhs=y_b[k][:],
                                     start=(k == 0), stop=(k == c.BT - 1))
                z = tmp_p.tile([128, c.EMB], F32, tag=f"z{bt}")
                nc.vector.tensor_copy(z[:], psz[:])
                sq = tmp_p.tile([128, c.EMB], F32, tag="sq")
                nc.vector.tensor_mul(sq[:], z[:], z[:])
                ss = tmp_p.tile([128, 1], F32, tag="ss")
                nc.vector.tensor_reduce(ss[:], sq[:], mybir.AxisListType.X,
                                        mybir.AluOpType.add)
                nrm = tmp_p.tile([128, 1], F32, tag="nrm")
                nc.scalar.sqrt(nrm[:], ss[:])
                nc.vector.tensor_scalar_max(nrm[:], nrm[:], 1e-12)
                rn = tmp_p.tile([128, 1], F32, tag="rn")
                nc.vector.reciprocal(rn[:], nrm[:])
                zn = tmp_p.tile([128, c.EMB], F32, tag=f"zn{bt}")
                nc.scalar.mul(zn[:], z[:], rn[:])
                nc.vector.tensor_add(accs[bt][:], accs[bt][:], zn[:])
                if layer == 0:
                    pse = tps2.tile([c.EMB, 128], F32, tag="tpe")
                    nc.tensor.transpose(out=pse[:], in_=zn[:],
                                        identity=ident[:, :])
                    nc.vector.tensor_copy(cur_e[:, bt * 128:(bt + 1) * 128],
                                          pse[:])

        for bt in range(c.BT):
            ot = tmp_p.tile([128, c.EMB], F32, tag="ot")
            nc.scalar.mul(ot[:], accs[bt][:], 1.0 / 3.0)
            nc.sync.dma_start(out_t[bt * 128:(bt + 1) * 128, :], ot[:])


# ---------------------------------------------------------------------------

def run_on_hw(cfg, plan, nc, in_maps):
    from concourse.bass_utils import run_bass_kernel_spmd
    res = run_bass_kernel_spmd(nc, in_maps, core_ids=list(range(cfg.M)))
    return res


def kernel(**inputs):
    cfg = Cfg()
    plan, in_maps = prep(cfg, inputs)
    nc = build_program(cfg, plan)
    res = run_on_hw(cfg, plan, nc, in_maps)
    out = np.asarray(res.results[0]["out"], np.float32)
    return out



# revision 7
# speedup vs baseline: 28.2346x; 28.2346x over previous
"""Trainium2 Bass kernel for COTREC-style GNN message passing.

Math (reference):
    cur1 = S @ emb                      (S sparse [N,N], 1M nnz)
    cur2 = S @ cur1
    item = (emb + cur1 + cur2) / 3
    sess = meanpool_sessions(item)      ([B, E])
    ... small dense tail (DA @ ..., w_sess, l2norm) ...

Device decomposition (8 cores, SPMD single program, per-core data via inputs):
  * The embedding table is uploaded bf16, row-sharded (one shard per core),
    and AllGathered on device into a full DRAM copy per core - 8x less
    host->device traffic than replicating it.
  * Row-shard nodes: core m owns rows [m*NLOC, (m+1)*NLOC).
  * cur1 shard computed locally: gather emb[src] per edge (one batched
    indirect DMA per chunk of windows; int32 indices, OOB pad slots skipped
    via bounds_check), segment-sum by destination row via mask-matmuls:
    fixed windows of W rows <-> one 128-slot tile; mask [128, W] carries the
    edge values (host-built, bf16).
  * cur2 never materialized. The session pooling is pushed through the
    graph: sess*3*len = P01@emb + P01@cur1 + (P01@S)@cur1, where P01/Q01
    structures are host-computed integer index work. Per-core partial
    pooled sums [E, B] -> one small AllReduce.
  * Dense tail replicated on all cores (DA = D @ A precomputed on host,
    shipped bf16); core 0's output returned.
"""

import os
import sys
from contextlib import ExitStack

import numpy as np
import ml_dtypes

BF16NP = ml_dtypes.bfloat16

for _p in ("/opt/trn_rl_repo", os.path.expanduser("~/.axon_site/_ro/trn_rl_repo")):
    if os.path.isdir(_p) and _p not in sys.path:
        sys.path.append(_p)

import concourse.bacc as bacc
import concourse.bass as bass
import concourse.tile as tile
from concourse import mybir
from concourse.masks import make_identity

F32 = mybir.dt.float32
BF16 = mybir.dt.bfloat16
I32 = mybir.dt.int32


class Cfg:
    def __init__(self, N=100000, NNZ=1000000, B=512, L=50, M=8, EMB=112,
                 W=8, WS=8):
        self.N, self.NNZ, self.B, self.L, self.M = N, NNZ, B, L, M
        self.EMB = EMB
        self.W = W            # output rows per L1 window (one 128-slot tile)
        self.WS = WS          # sessions per sess window
        # shard rows per core: multiple of 128, cover N
        self.NLOC = ((N + M - 1) // M + 127) // 128 * 128
        self.NTOT = self.NLOC * M          # rows in allgathered table
        self.G = self.NLOC // 128          # 128-row groups per core
        self.NWIN = self.NLOC // W         # L1 windows per core
        self.TPG = 128 // W                # L1 tiles (windows) per group
        self.NWS = B // WS                 # sess windows
        self.BT = B // 128                 # tail b-tiles


# ---------------------------------------------------------------------------
# Host preprocessing: pure integer/layout work + permutation of input floats.
# ---------------------------------------------------------------------------

def _csr_expand(rowptr, rows):
    """For each r in rows return concatenated [rowptr[r], rowptr[r+1]) ranges."""
    deg = rowptr[rows + 1] - rowptr[rows]
    total = int(deg.sum())
    if total == 0:
        return np.zeros(0, np.int64), deg
    cum = np.cumsum(deg)
    out = np.arange(total, dtype=np.int64) - np.repeat(cum - deg, deg) \
        + np.repeat(rowptr[rows], deg)
    return out, deg


def prep(cfg, inputs):
    """Build per-core input arrays + the (core-independent) program plan."""
    c = cfg
    emb = np.asarray(inputs["embedding"], np.float32)
    av = np.asarray(inputs["adj_vals"], np.float32)
    ar = np.asarray(inputs["adj_rows"], np.int64)
    ac = np.asarray(inputs["adj_cols"], np.int64)
    D = np.asarray(inputs["D"], np.float32)
    A = np.asarray(inputs["A"], np.float32)
    si = np.asarray(inputs["session_item"], np.int64)
    sl = np.asarray(inputs["session_len"], np.float32)
    w_sess = np.asarray(inputs["w_sess"], np.float32)

    # bf16 sharded embedding table (padded with zero rows to NTOT)
    emb16 = np.zeros((c.NTOT, c.EMB), BF16NP)
    emb16[:c.N] = emb.astype(BF16NP)

    # session refs: (b, col) for non-pad items
    b_ref = np.repeat(np.arange(c.B, dtype=np.int64), c.L)
    it_ref = si.ravel()
    keep = it_ref > 0
    b_ref, col_ref = b_ref[keep], it_ref[keep] - 1       # cols in [0, N)

    # CSR of S by row (for Q01 = P01 @ S)
    order = np.argsort(ar, kind="stable")
    ar_s, ac_s, av_s = ar[order], ac[order], av[order]
    rowptr = np.searchsorted(ar_s, np.arange(c.N + 1)).astype(np.int64)
    epos, deg = _csr_expand(rowptr, col_ref)
    q_b = np.repeat(b_ref, deg)
    q_c = ac_s[epos]
    q_v = av_s[epos]

    # prune L1 edges to rows actually referenced by T1/T2
    ref_mask = np.zeros(c.N, bool)
    ref_mask[col_ref] = True
    ref_mask[q_c] = True
    ekeep = ref_mask[ar]
    er, ec, ev = ar[ekeep], ac[ekeep], av[ekeep]

    own_e = er // c.NLOC
    own_ref = col_ref // c.NLOC
    own_q = q_c // c.NLOC

    # ---- L1 per-core window fill ------------------------------------------
    def l1_core(m):
        sel = own_e == m
        r = er[sel] - m * c.NLOC
        cc, vv = ec[sel], ev[sel]
        w = r // c.W
        o = r - w * c.W
        so = np.argsort(w, kind="stable")
        w, o, cc, vv = w[so], o[so], cc[so], vv[so]
        cnt = np.bincount(w, minlength=c.NWIN)
        starts = np.zeros(c.NWIN, np.int64)
        starts[1:] = np.cumsum(cnt)[:-1]
        slot = np.arange(len(w)) - starts[w]
        return w, o, cc, vv, slot, cnt

    l1 = [l1_core(m) for m in range(c.M)]
    l1_max = max(int(x[5].max()) if x[5].size else 0 for x in l1)
    if l1_max > 128:
        raise RuntimeError(f"L1 window overflow: {l1_max} > 128; reduce W")

    l1_idx = np.full((c.M, 128, c.NWIN), c.N, np.int32)      # OOB pad = N
    l1_msk = np.zeros((c.M, 128, c.NWIN, c.W), BF16NP)
    for m, (w, o, cc, vv, slot, _) in enumerate(l1):
        l1_idx[m, slot, w] = cc
        l1_msk[m, slot, w, o] = vv.astype(BF16NP)

    # ---- sess stream A: P01 @ emb (gathers emb at col_ref) ----------------
    # ---- sess stream B: (P01 + Q01) @ cur1 (gathers local cur1 rows) ------
    def sess_core_stream(m, bb, cc_local, vv):
        w = bb // c.WS
        o = bb - w * c.WS
        so = np.argsort(w, kind="stable")
        w, o, cc_local, vv = w[so], o[so], cc_local[so], vv[so]
        cnt = np.bincount(w, minlength=c.NWS)
        return w, o, cc_local, vv, cnt

    sa = []
    sb = []
    for m in range(c.M):
        selr = own_ref == m
        sa.append(sess_core_stream(m, b_ref[selr], col_ref[selr],
                                   np.ones(int(selr.sum()), np.float32)))
        selq = own_q == m
        bb = np.concatenate([b_ref[selr], q_b[selq]])
        cl = np.concatenate([col_ref[selr] - m * c.NLOC, q_c[selq] - m * c.NLOC])
        vv = np.concatenate([np.ones(int(selr.sum()), np.float32), q_v[selq]])
        sb.append(sess_core_stream(m, bb, cl, vv))

    def stream_caps(streams):
        cnts = np.stack([s[4] for s in streams])       # [M, NWS]
        mx = cnts.max(axis=0)
        caps = np.maximum(128, ((mx + 127) // 128) * 128).astype(np.int64)
        return caps

    sa_caps = stream_caps(sa)
    sb_caps = stream_caps(sb)

    def stream_fill(streams, caps, oob):
        tiles_w = caps // 128                           # tiles per window
        tbase = np.zeros(c.NWS, np.int64)
        tbase[1:] = np.cumsum(tiles_w)[:-1]
        T = int(tiles_w.sum())
        wmap = np.zeros(T, np.int64)
        for wi in range(c.NWS):
            wmap[tbase[wi]:tbase[wi] + tiles_w[wi]] = wi
        idx = np.full((c.M, 128, T), oob, np.int32)
        msk = np.zeros((c.M, 128, T, c.WS), BF16NP)
        for m, (w, o, cl, vv, cnt) in enumerate(streams):
            starts = np.zeros(c.NWS, np.int64)
            starts[1:] = np.cumsum(cnt)[:-1]
            slot = np.arange(len(w)) - starts[w]        # slot within window
            t = tbase[w] + slot // 128
            p = slot % 128
            idx[m, p, t] = cl
            msk[m, p, t, o] = vv.astype(BF16NP)
        return idx, msk, wmap, T

    sa_idx, sa_msk, sa_wmap, saT = stream_fill(sa, sa_caps, c.N)
    sb_idx, sb_msk, sb_wmap, sbT = stream_fill(sb, sb_caps, c.NLOC)

    # session_len layout for per-partition scale: lenr[p, i] = len[128*i + p]
    lenr = sl.reshape(c.BT, 128).T.astype(np.float32).copy()

    da_t = np.ascontiguousarray((D @ A).T.astype(BF16NP))    # DA^T, bf16
    wt = np.stack([w_sess[i].T for i in range(w_sess.shape[0])]).astype(BF16NP)

    in_maps = []
    for m in range(c.M):
        in_maps.append({
            "emb_sh": np.ascontiguousarray(emb16[m * c.NLOC:(m + 1) * c.NLOC]),
            "l1_idx": np.ascontiguousarray(l1_idx[m]),
            "l1_msk": np.ascontiguousarray(l1_msk[m]),
            "sa_idx": np.ascontiguousarray(sa_idx[m]),
            "sa_msk": np.ascontiguousarray(sa_msk[m]),
            "sb_idx": np.ascontiguousarray(sb_idx[m]),
            "sb_msk": np.ascontiguousarray(sb_msk[m]),
            "da_t": da_t,
            "wt": wt,
            "lenr": lenr,
        })

    plan = {"saT": saT, "sbT": sbT,
            "sa_wmap": sa_wmap.tolist(), "sb_wmap": sb_wmap.tolist()}
    return plan, in_maps


# ---------------------------------------------------------------------------
# Bass program (identical on all cores; per-core behavior comes from inputs)
# ---------------------------------------------------------------------------

CHUNK_TILES = 32


def _chunks(total, size):
    out = []
    s = 0
    while s < total:
        out.append((s, min(size, total - s)))
        s += size
    return out


def build_program(cfg, plan):
    c = cfg
    nc = bacc.Bacc("TRN2", target_bir_lowering=False, debug=False,
                   num_devices=c.M)

    emb_sh_t = nc.dram_tensor("emb_sh", [c.NLOC, c.EMB], BF16, kind="ExternalInput")
    l1_idx_t = nc.dram_tensor("l1_idx", [128, c.NWIN], I32, kind="ExternalInput")
    l1_msk_t = nc.dram_tensor("l1_msk", [128, c.NWIN, c.W], BF16, kind="ExternalInput")
    sa_idx_t = nc.dram_tensor("sa_idx", [128, plan["saT"]], I32, kind="ExternalInput")
    sa_msk_t = nc.dram_tensor("sa_msk", [128, plan["saT"], c.WS], BF16, kind="ExternalInput")
    sb_idx_t = nc.dram_tensor("sb_idx", [128, plan["sbT"]], I32, kind="ExternalInput")
    sb_msk_t = nc.dram_tensor("sb_msk", [128, plan["sbT"], c.WS], BF16, kind="ExternalInput")
    da_t_t = nc.dram_tensor("da_t", [c.B, c.B], BF16, kind="ExternalInput")
    wt_t = nc.dram_tensor("wt", [2, c.EMB, c.EMB], BF16, kind="ExternalInput")
    lenr_t = nc.dram_tensor("lenr", [128, c.BT], F32, kind="ExternalInput")
    out_t = nc.dram_tensor("out", [c.B, c.EMB], F32, kind="ExternalOutput")

    emb_loc_t = nc.dram_tensor("emb_loc", [c.NLOC, c.EMB], BF16,
                               kind="Internal")
    emb_full_t = nc.dram_tensor("emb_full", [c.NTOT, c.EMB], BF16,
                                kind="Internal", addr_space="Shared")
    cur1_t = nc.dram_tensor("cur1", [c.NLOC, c.EMB], BF16, kind="Internal")
    ar_in_t = nc.dram_tensor("ar_in", [c.EMB, c.B], F32, kind="Internal")
    ar_out_t = nc.dram_tensor("ar_out", [c.EMB, c.B], F32, kind="Internal",
                              addr_space="Shared")

    with tile.TileContext(nc) as tc, ExitStack() as ctx:
        _body(ctx, tc, c, plan, emb_sh_t, emb_loc_t, emb_full_t, l1_idx_t,
              l1_msk_t, sa_idx_t, sa_msk_t, sb_idx_t, sb_msk_t, da_t_t, wt_t,
              lenr_t, out_t, cur1_t, ar_in_t, ar_out_t)

    nc.compile()
    return nc


def _body(ctx, tc, c, plan, emb_sh_t, emb_loc_t, emb_full_t, l1_idx_t,
          l1_msk_t, sa_idx_t, sa_msk_t, sb_idx_t, sb_msk_t, da_t_t, wt_t,
          lenr_t, out_t, cur1_t, ar_in_t, ar_out_t):
    nc = tc.nc
    CT = CHUNK_TILES

    const_p = ctx.enter_context(tc.tile_pool(name="const", bufs=1))
    ident = const_p.tile([128, 128], F32)
    make_identity(nc, ident[:])

    # ------------- phase 0: assemble the full bf16 table on device ---------
    # collectives cannot read IO tensors -> stage shard into Internal DRAM
    nc.sync.dma_start(emb_loc_t[:], emb_sh_t[:])
    nc.gpsimd.collective_compute(
        "AllGather", mybir.AluOpType.bypass,
        replica_groups=[list(range(c.M))],
        ins=[emb_loc_t.ap().opt()], outs=[emb_full_t.ap().opt()])

    # persistent gather chunk buffers (manual double buffer, memset once so
    # OOB-skipped slots never expose NaN garbage to the matmul)
    gb_p = ctx.enter_context(tc.tile_pool(name="gbuf", bufs=1))
    gbufs = [gb_p.tile([128, CT, c.EMB], BF16, tag=f"gb{i}", name=f"gb{i}")
             for i in range(2)]
    for t in gbufs:
        nc.vector.memset(t[:], 0.0)

    sess_ps_p = ctx.enter_context(tc.tile_pool(name="sessps", bufs=1, space="PSUM"))
    sess_ps = sess_ps_p.tile([c.EMB, c.B], F32)

    gb_i = 0

    # ---------------- phase 1: cur1 = S @ emb (local row shard) ------------
    with tc.tile_pool(name="l1mi", bufs=3) as mi_p, \
         tc.tile_pool(name="l1ps", bufs=3, space="PSUM") as ps_p, \
         tc.tile_pool(name="l1tp", bufs=2, space="PSUM") as tp_p, \
         tc.tile_pool(name="l1st", bufs=3) as st_p:
        for ch_start, ch_n in _chunks(c.NWIN, CT):
            idx_sb = mi_p.tile([128, CT], I32, tag="idx")
            nc.sync.dma_start(idx_sb[:, :ch_n],
                              l1_idx_t[:, ch_start:ch_start + ch_n])
            msk_sb = mi_p.tile([128, CT, c.W], BF16, tag="msk")
            nc.sync.dma_start(msk_sb[:, :ch_n, :],
                              l1_msk_t[:, ch_start:ch_start + ch_n, :])
            gb = gbufs[gb_i % 2]
            gb_i += 1
            for tt in range(ch_n):
                nc.gpsimd.indirect_dma_start(
                    out=gb[:, tt, :], out_offset=None,
                    in_=emb_full_t[:],
                    in_offset=bass.IndirectOffsetOnAxis(
                        ap=idx_sb[:, tt:tt + 1], axis=0),
                    bounds_check=c.N - 1, oob_is_err=False)

            assert ch_start % c.TPG == 0 and ch_n % c.TPG == 0
            for gl in range(ch_n // c.TPG):
                g = (ch_start // c.TPG) + gl
                ps = ps_p.tile([c.EMB, 128], F32, tag="ps")
                for j in range(c.TPG):
                    t = gl * c.TPG + j
                    nc.tensor.matmul(
                        out=ps[:, j * c.W:(j + 1) * c.W],
                        lhsT=gb[:, t, :],
                        rhs=msk_sb[:, t, :],
                        start=(j == 0), stop=(j == c.TPG - 1))
                s1 = st_p.tile([c.EMB, 128], F32, tag="s1")
                nc.vector.tensor_copy(s1[:], ps[:])
                ps2 = tp_p.tile([128, c.EMB], F32, tag="tp")
                nc.tensor.transpose(out=ps2[:], in_=s1[:],
                                    identity=ident[:c.EMB, :c.EMB])
                s2 = st_p.tile([128, c.EMB], BF16, tag="s2")
                nc.vector.tensor_copy(s2[:], ps2[:])
                nc.sync.dma_start(cur1_t[g * 128:(g + 1) * 128, :], s2[:])

    # ---------------- phase 2: pooled partial sums into sess_ps ------------
    streams = [
        (sa_idx_t, sa_msk_t, plan["saT"], plan["sa_wmap"], emb_full_t, c.N),
        (sb_idx_t, sb_msk_t, plan["sbT"], plan["sb_wmap"], cur1_t, c.NLOC),
    ]
    n_sess_mm = plan["saT"] + plan["sbT"]
    mm_i = 0
    with tc.tile_pool(name="smi", bufs=3) as mi_p:
        for idx_t_, msk_t_, T, wmap, src_t, nrows in streams:
            for ch_start, ch_n in _chunks(T, CT):
                idx_sb = mi_p.tile([128, CT], I32, tag="idx")
                nc.sync.dma_start(idx_sb[:, :ch_n],
                                  idx_t_[:, ch_start:ch_start + ch_n])
                msk_sb = mi_p.tile([128, CT, c.WS], BF16, tag="msk")
                nc.sync.dma_start(msk_sb[:, :ch_n, :],
                                  msk_t_[:, ch_start:ch_start + ch_n, :])
                gb = gbufs[gb_i % 2]
                gb_i += 1
                for tt in range(ch_n):
                    nc.gpsimd.indirect_dma_start(
                        out=gb[:, tt, :], out_offset=None,
                        in_=src_t[:],
                        in_offset=bass.IndirectOffsetOnAxis(
                            ap=idx_sb[:, tt:tt + 1], axis=0),
                        bounds_check=nrows - 1, oob_is_err=False)
                for t in range(ch_n):
                    w = wmap[ch_start + t]
                    nc.tensor.matmul(
                        out=sess_ps[:, w * c.WS:(w + 1) * c.WS],
                        lhsT=gb[:, t, :],
                        rhs=msk_sb[:, t, :],
                        start=(mm_i == 0), stop=(mm_i == n_sess_mm - 1))
                    mm_i += 1

    # ---------------- phase 3: AllReduce + dense tail ----------------------
    with tc.tile_pool(name="tail", bufs=1) as tp, \
         tc.tile_pool(name="tailps", bufs=1, space="PSUM") as tps, \
         tc.tile_pool(name="tailps2", bufs=1, space="PSUM") as tps2, \
         tc.tile_pool(name="tmp", bufs=2) as tmp_p:
        sess_sb = tp.tile([c.EMB, c.B], F32, tag="sess_sb")
        nc.vector.tensor_copy(sess_sb[:], sess_ps[:])
        nc.sync.dma_start(ar_in_t[:], sess_sb[:])
        nc.gpsimd.collective_compute(
            "AllReduce", mybir.AluOpType.add,
            replica_groups=[list(range(c.M))],
            ins=[ar_in_t.ap().opt()], outs=[ar_out_t.ap().opt()])
        sess_all = tp.tile([c.EMB, c.B], F32, tag="sess_all")
        nc.sync.dma_start(sess_all[:], ar_out_t[:])

        lr = tp.tile([128, c.BT], F32, tag="lr")
        nc.sync.dma_start(lr[:], lenr_t[:])
        rc3 = tp.tile([128, c.BT], F32, tag="rc3")
        nc.vector.reciprocal(rc3[:], lr[:])
        nc.vector.tensor_scalar_mul(rc3[:], rc3[:], 1.0 / 3.0)

        # sess0 b-tiles (scaled) + acc + back-transpose to e-layout
        accs = [tp.tile([128, c.EMB], F32, tag=f"acc{i}", name=f"acc{i}")
                for i in range(c.BT)]
        cur_e = tp.tile([c.EMB, c.B], BF16, tag="cur_e0")
        for i in range(c.BT):
            pst = tps.tile([128, c.EMB], F32, tag="tp")
            nc.tensor.transpose(out=pst[:],
                                in_=sess_all[:, i * 128:(i + 1) * 128],
                                identity=ident[:c.EMB, :c.EMB])
            s0 = tmp_p.tile([128, c.EMB], F32, tag="s0")
            nc.scalar.mul(s0[:], pst[:], rc3[:, i:i + 1])
            nc.vector.tensor_copy(accs[i][:], s0[:])
            pse = tps2.tile([c.EMB, 128], F32, tag="tpe")
            nc.tensor.transpose(out=pse[:], in_=s0[:], identity=ident[:, :])
            nc.vector.tensor_copy(cur_e[:, i * 128:(i + 1) * 128], pse[:])

        # DA^T b'-tiles (host-precomputed, bf16)
        da_sb = [tp.tile([128, c.B], BF16, tag=f"da{i}", name=f"dasb{i}")
                 for i in range(c.BT)]
        for i in range(c.BT):
            nc.sync.dma_start(da_sb[i][:], da_t_t[i * 128:(i + 1) * 128, :])

        wt_sb = tp.tile([c.EMB, 2, c.EMB], BF16, tag="wt")
        for i in range(2):
            nc.sync.dma_start(wt_sb[:, i, :], wt_t[i])

        for layer in range(2):
            psy = tps.tile([c.EMB, c.B], F32, tag="ypsum")
            nc.tensor.matmul(out=psy[:], lhsT=wt_sb[:, layer, :], rhs=cur_e[:],
                             start=True, stop=True)
            y_e = tmp_p.tile([c.EMB, c.B], F32, tag="y_e")
            nc.vector.tensor_copy(y_e[:], psy[:])
            y_b = []
            for bt in range(c.BT):
                pst = tps.tile([128, c.EMB], F32, tag="tp")
                nc.tensor.transpose(out=pst[:],
                                    in_=y_e[:, bt * 128:(bt + 1) * 128],
                                    identity=ident[:c.EMB, :c.EMB])
                yb = tmp_p.tile([128, c.EMB], BF16, tag=f"yb{bt}")
                nc.vector.tensor_copy(yb[:], pst[:])
                y_b.append(yb)
            if layer == 0:
                cur_e = tp.tile([c.EMB, c.B], BF16, tag="cur_e1")
            for bt in range(c.BT):
                psz = tps.tile([128, c.EMB], F32, tag="zps")
                for k in range(c.BT):
                    nc.tensor.matmul(out=psz[:],
                                     lhsT=da_sb[k][:, bt * 128:(bt + 1) * 128],
                                     rhs=y_b[k][:],
                                     start=(k == 0), stop=(k == c.BT - 1))
                z = tmp_p.tile([128, c.EMB], F32, tag=f"z{bt}")
                nc.vector.tensor_copy(z[:], psz[:])
                sq = tmp_p.tile([128, c.EMB], F32, tag="sq")
                nc.vector.tensor_mul(sq[:], z[:], z[:])
                ss = tmp_p.tile([128, 1], F32, tag="ss")
                nc.vector.tensor_reduce(ss[:], sq[:], mybir.AxisListType.X,
                                        mybir.AluOpType.add)
                nrm = tmp_p.tile([128, 1], F32, tag="nrm")
                nc.scalar.sqrt(nrm[:], ss[:])
                nc.vector.tensor_scalar_max(nrm[:], nrm[:], 1e-12)
                rn = tmp_p.tile([128, 1], F32, tag="rn")
                nc.vector.reciprocal(rn[:], nrm[:])
                zn = tmp_p.tile([128, c.EMB], F32, tag=f"zn{bt}")
                nc.scalar.mul(zn[:], z[:], rn[:])
                nc.vector.tensor_add(accs[bt][:], accs[bt][:], zn[:])
                if layer == 0:
                    pse = tps2.tile([c.EMB, 128], F32, tag="tpe")
                    nc.tensor.transpose(out=pse[:], in_=zn[:],
                                        identity=ident[:, :])
                    nc.vector.tensor_copy(cur_e[:, bt * 128:(bt + 1) * 128],
                                          pse[:])

        for bt in range(c.BT):
            ot = tmp_p.tile([128, c.EMB], F32, tag="ot")
            nc.scalar.mul(ot[:], accs[bt][:], 1.0 / 3.0)
            nc.sync.dma_start(out_t[bt * 128:(bt + 1) * 128, :], ot[:])


# ---------------------------------------------------------------------------

def run_on_hw(cfg, plan, nc, in_maps):
    from concourse.bass_utils import run_bass_kernel_spmd
    res = run_bass_kernel_spmd(nc, in_maps, core_ids=list(range(cfg.M)))
    return res


def kernel(**inputs):
    cfg = Cfg()
    plan, in_maps = prep(cfg, inputs)
    nc = build_program(cfg, plan)
    res = run_on_hw(cfg, plan, nc, in_maps)
    out = np.asarray(res.results[0]["out"], np.float32)
    return out
